# revision 13
# baseline (speedup 1.0000x reference)
"""Attention-simple kernel for TRN2, 8 NeuronCores, data-parallel over batch.

Computation (per example b):
    h = tanh(context[b] @ W.T)          # [S, D]
    logits = h @ v (+ bias, dropped: softmax-invariant)
    attn = softmax(logits)              # [S]
    wc = attn @ context[b]              # [D]
returns (wc [B, D], attn [B, S])

Host-side prep (not counted in HW time): shard batch 4-per-core, cast to
bf16, and pre-transpose context to [D, S] so the PE contraction dim (d)
lands on SBUF partitions without any on-chip transposes.

Per-core on-chip pipeline, per batch:
  - ctxT tiles [128d, 512s] arrive via DMA; W.T resident in SBUF.
  - pre[s,e] accumulated over 8 d-chunks into PSUM ([128s, 512e] x2).
  - ACT applies tanh PSUM->SBUF (bf16).
  - One DVE tensor_tensor_reduce: prod = h*v_bcast, accum_out = logits col.
  - softmax without max-subtraction (logits are tanh-bounded, |l| < ~6):
    ACT exp with fused accum -> per-partition sums; PE ones-matmul -> total;
    DVE reciprocal; PE broadcast matmul -> inv on all partitions.
  - weighted sum: PE accumulates exp(l) chunks against natural-layout ctx
    tiles, then DVE scales by inv(sum).
"""

import numpy as np
import ml_dtypes

import concourse.bass as bass
import concourse.mybir as mybir
import concourse.tile as tile
from concourse.bass_utils import run_bass_kernel_spmd


# ---------------------------------------------------------------------------
# This walrus build accepts at most ONE sync-wait command per instruction,
# while Tile emits several (e.g. the kernel-tail Drain waits on every sem
# lane). Hoist excess waits onto injected wait-only EventSemaphore
# instructions on the same engine, immediately before the instruction —
# strictly stronger ordering, so always safe.
import json as _json
import concourse.bass2jax as _b2j

_MAX_WAITS = 1
_orig_compile_bir_kernel = _b2j.compile_bir_kernel


def _split_excess_waits(bir, max_waits=_MAX_WAITS):
    bj = _json.loads(bir)
    changed = False
    for f in bj.get("functions", []):
        for bb in (f.get("basicblocks") or f.get("blocks") or []):
            out = []
            for ins in bb.get("instructions", []):
                si = ins.get("sync_info")
                if si:
                    w = si.get("on_wait") or []
                    if len(w) > max_waits:
                        for i, x in enumerate(w[:-max_waits]):
                            stub = {
                                "name": f"{ins['name']}-ws{i}",
                                "opcode": "EventSemaphore",
                                "engine": ins["engine"],
                                "ins": [],
                                "outs": [],
                                "sync_info": {"on_wait": [x], "on_update": []},
                            }
                            if "debug" in ins:
                                stub["debug"] = ins["debug"]
                            out.append(stub)
                        si["on_wait"] = w[-max_waits:]
                        changed = True
                out.append(ins)
            bb["instructions"] = out
    if not changed:
        return bir
    s = _json.dumps(bj)
    return s.encode() if isinstance(bir, bytes) else s


def _patched_compile_bir_kernel(bir_json, tmpdir, neff_name="file.neff"):
    return _orig_compile_bir_kernel(_split_excess_waits(bir_json), tmpdir, neff_name)


if _b2j.compile_bir_kernel is not _patched_compile_bir_kernel:
    _b2j.compile_bir_kernel = _patched_compile_bir_kernel
# ---------------------------------------------------------------------------

BF16 = mybir.dt.bfloat16
F32 = mybir.dt.float32
AF = mybir.ActivationFunctionType
ALU = mybir.AluOpType

B, S, D = 32, 2048, 1024
N_CORES = 8
BPC = B // N_CORES       # batches per core
P = 128
S_TILES = S // P         # 16
S_GROUPS = 4             # ctxT DMA groups per batch
SG = S // S_GROUPS       # 512
DC = D // P              # 8 contraction chunks
EH = 2                   # e halves of 512
NF = 512                 # matmul free dim / psum bank


def build_nc():
    nc = bass.Bass()
    ctxT = nc.declare_dram_parameter("ctxT", [BPC, D, S], BF16, isOutput=False)
    nat = nc.declare_dram_parameter("nat", [BPC, S, D], BF16, isOutput=False)
    wt = nc.declare_dram_parameter("wt", [D, D], BF16, isOutput=False)
    vb = nc.declare_dram_parameter("vb", [P, D], F32, isOutput=False)
    ones = nc.declare_dram_parameter("ones", [P, P], F32, isOutput=False)
    ident = nc.declare_dram_parameter("ident", [P, P], F32, isOutput=False)
    wc_out = nc.declare_dram_parameter("wc", [BPC, D], F32, isOutput=True)
    attn_out = nc.declare_dram_parameter("attn", [BPC, S], F32, isOutput=True)

    with tile.TileContext(nc) as tc:
        with (
            tc.tile_pool(name="const", bufs=1) as cpool,
            tc.tile_pool(name="ctxt", bufs=4) as ctxt_pool,
            tc.tile_pool(name="hbuf", bufs=3) as hpool,
            tc.tile_pool(name="prod", bufs=2) as prod_pool,
            tc.tile_pool(name="wsum", bufs=18) as wsum_pool,
            tc.tile_pool(name="small", bufs=2) as small_pool,
            tc.tile_pool(name="psum_pre", bufs=3, space="PSUM") as psum_pre,
            tc.tile_pool(name="psum_wc", bufs=2, space="PSUM") as psum_wc_pool,
            tc.tile_pool(name="psum_sm", bufs=1, space="PSUM") as psum_sm,
        ):
            wt_sb = cpool.tile([P, DC, D], BF16)
            wt_r = wt.rearrange("(dc p) e -> p dc e", p=P)

            # First ctxT group split per d-chunk and interleaved with the wt
            # chunks in consumption order, so MM(dc=0) can start after ~384KB
            # of DMA instead of 3MB.
            ctxT_first = ctxt_pool.tile([P, DC, SG], BF16, tag="ctxT_g")
            ctxT_first_r = ctxT[0, :, 0:SG].rearrange("(dc p) s -> p dc s", p=P)
            for dc in range(DC):
                nc.sync.dma_start(
                    ctxT_first[:, dc:dc + 1, :], ctxT_first_r[:, dc:dc + 1, :]
                )
                nc.scalar.dma_start(wt_sb[:, dc:dc + 1, :], wt_r[:, dc:dc + 1, :])
            vb_sb = cpool.tile([P, D], F32)
            nc.scalar.dma_start(vb_sb, vb[:])
            ones_sb = cpool.tile([P, P], F32)
            nc.scalar.dma_start(ones_sb, ones[:])
            ident_sb = cpool.tile([P, P], F32)
            nc.scalar.dma_start(ident_sb, ident[:])
            invpad = cpool.tile([P, BPC], F32)
            nc.gpsimd.memset(invpad, 0.0)

            # Small floor-cost PE warm-up that finishes before the first real
            # matmul's data lands: pushes HAM toward 8/8 without delaying the
            # real stream behind the FIFO queue.
            warm_src = cpool.tile([P, P], BF16)
            nc.vector.memset(warm_src, 0.0)
            warm_ps = psum_sm.tile([P, NF], F32, tag="sm")
            for _ in range(30):
                nc.tensor.matmul(warm_ps[:, 0:64], warm_src[:, 0:P], warm_src[:, 0:64])

            # Online weighted sum: exp of each logits column is taken as soon
            # as that s-tile's logits land, cast to bf16, and its two wsum
            # matmuls accumulate into the batch PSUM. The wsum matmuls are
            # emitted with a 2-tile delay so the PE never stalls on the
            # tanh->dot->exp->cast chain.
            WSUM_DELAY = 2

            per_batch = {}     # b -> (logits, E, E16, wcps)
            pending = []       # [(b, t)] wsum work not yet emitted

            def emit_wsum(b, t):
                _, _, E16, wcps = per_batch[b]
                nat_t = wsum_pool.tile([P, D], BF16, tag="nat")
                nc.sync.dma_start(nat_t, nat[b, t * P:(t + 1) * P, :])
                for eh in range(EH):
                    nc.tensor.matmul(
                        wcps[0:1, eh * NF:(eh + 1) * NF],
                        E16[:, t:t + 1],
                        nat_t[:, eh * NF:(eh + 1) * NF],
                        start=(t == 0),
                        stop=(t == S_TILES - 1),
                    )

            def drain_pending(keep):
                while len(pending) > keep:
                    emit_wsum(*pending.pop(0))

            def pre_tile(b, t, ctxT_g, t4):
                logits, E, E16, _ = per_batch[b]
                ps = [
                    psum_pre.tile([P, NF], F32, tag="pre", name=f"pre_{b}_{t}_{eh}")
                    for eh in range(EH)
                ]
                for dc in range(DC):
                    lhsT = ctxT_g[:, dc, t4 * P:(t4 + 1) * P]
                    for eh in range(EH):
                        nc.tensor.matmul(
                            ps[eh],
                            lhsT,
                            wt_sb[:, dc, eh * NF:(eh + 1) * NF],
                            start=(dc == 0),
                            stop=(dc == DC - 1),
                        )
                h32 = hpool.tile([P, D], F32, tag="h32")
                for eh in range(EH):
                    nc.scalar.activation(
                        h32[:, eh * NF:(eh + 1) * NF], ps[eh], AF.Tanh
                    )
                prod = prod_pool.tile([P, D], F32, tag="prod")
                # accum_out sums (h * v) in fp32 before `prod` rounding
                nc.vector.scalar_tensor_tensor(
                    out=prod,
                    in0=h32,
                    scalar=1.0,
                    in1=vb_sb,
                    op0=ALU.mult,
                    op1=ALU.mult,
                    accum_out=logits[:, t:t + 1],
                )
                nc.scalar.activation(
                    E[:, t:t + 1], logits[:, t:t + 1], AF.Exp
                )
                nc.vector.tensor_copy(E16[:, t:t + 1], E[:, t:t + 1])
                pending.append((b, t))
                drain_pending(WSUM_DELAY)

            def finalize(b):
                _, E, E16, wcps = per_batch[b]
                sumexp = small_pool.tile([P, 1], F32, tag="sumexp")
                nc.vector.reduce_sum(sumexp, E, axis=mybir.AxisListType.X)
                smps = psum_sm.tile([P, NF], F32, tag="sm")
                # total = sum over partitions of sumexp  -> smps[0,0]
                nc.tensor.matmul(smps[0:1, 0:1], sumexp, ones_sb[:, 0:1])
                nc.vector.reciprocal(invpad[0:1, b:b + 1], smps[0:1, 0:1])
                # broadcast inv to all 128 partitions -> smps[:,1]
                nc.tensor.matmul(smps[:, 1:2], ones_sb, invpad[:, b:b + 1])
                attn_f = small_pool.tile([P, S_TILES], F32, tag="attn_f")
                nc.vector.tensor_mul(
                    out=attn_f,
                    in0=E,
                    in1=smps[:, 1:2].to_broadcast([P, S_TILES]),
                )
                nc.tensor.transpose(smps[0:S_TILES, P:2 * P], attn_f, ident_sb)
                attnT = small_pool.tile([S_TILES, P], F32, tag="attnT")
                nc.vector.tensor_copy(attnT, smps[0:S_TILES, P:2 * P])
                nc.sync.dma_start(
                    attn_out[b].rearrange("(t p) -> t p", p=P), attnT
                )
                wc_sb = small_pool.tile([1, D], F32, tag="wc_sb")
                nc.vector.tensor_scalar_mul(wc_sb, wcps, invpad[0:1, b:b + 1])
                nc.sync.dma_start(wc_out[b][None, :], wc_sb)
                del per_batch[b]

            for b in range(BPC):
                per_batch[b] = (
                    small_pool.tile([P, S_TILES], F32, tag="logits", name=f"lg{b}"),
                    small_pool.tile([P, S_TILES], F32, tag="E", name=f"E{b}"),
                    small_pool.tile([P, S_TILES], BF16, tag="E16", name=f"e16_{b}"),
                    psum_wc_pool.tile([1, D], F32, tag="wc", name=f"wc{b}"),
                )
                for g in range(S_GROUPS):
                    if b == 0 and g == 0:
                        ctxT_g = ctxT_first
                    else:
                        ctxT_g = ctxt_pool.tile([P, DC, SG], BF16, tag="ctxT_g")
                        nc.sync.dma_start(
                            ctxT_g,
                            ctxT[b, :, g * SG:(g + 1) * SG].rearrange(
                                "(dc p) s -> p dc s", p=P
                            ),
                        )
                    for t4 in range(SG // P):
                        pre_tile(b, g * (SG // P) + t4, ctxT_g, t4)
                if b > 0:
                    finalize(b - 1)
            drain_pending(0)
            finalize(BPC - 1)

    return nc


def _prep_in_maps(context, W, v):
    ctx16 = np.asarray(context, dtype=np.float32).astype(ml_dtypes.bfloat16)
    wt16 = np.ascontiguousarray(np.asarray(W, dtype=np.float32).T).astype(
        ml_dtypes.bfloat16
    )
    v32 = np.asarray(v, dtype=np.float32)
    vb32 = np.ascontiguousarray(np.broadcast_to(v32[None, :], (P, D)))
    ones = np.ones((P, P), dtype=np.float32)
    ident = np.eye(P, dtype=np.float32)

    in_maps = []
    for c in range(N_CORES):
        shard = ctx16[c * BPC:(c + 1) * BPC]            # [BPC, S, D] bf16
        ctxT = np.ascontiguousarray(shard.transpose(0, 2, 1))  # [BPC, D, S]
        in_maps.append(
            {
                "ctxT": ctxT,
                "nat": np.ascontiguousarray(shard),
                "wt": wt16,
                "vb": vb32,
                "ones": ones,
                "ident": ident,
            }
        )
    return in_maps


def run(context, W, v, b=None, trace=False, **trace_kwargs):
    nc = build_nc()
    in_maps = _prep_in_maps(context, W, v)
    res = run_bass_kernel_spmd(
        nc, in_maps, list(range(N_CORES)), trace=trace, **trace_kwargs
    )
    wc = np.concatenate([r["wc"] for r in res.results], axis=0)
    attn = np.concatenate([r["attn"] for r in res.results], axis=0)
    return (wc, attn), res


def kernel(context, W, v, b=None, **_ignored):
    out, _ = run(context, W, v, b)
    return out


# revision 14
# speedup vs baseline: 1.0152x; 1.0152x over previous
"""Attention-simple kernel for TRN2, 8 NeuronCores, data-parallel over batch.

Computation (per example b):
    h = tanh(context[b] @ W.T)          # [S, D]
    logits = h @ v (+ bias, dropped: softmax-invariant)
    attn = softmax(logits)              # [S]
    wc = attn @ context[b]              # [D]
returns (wc [B, D], attn [B, S])

Host-side prep (not counted in HW time): shard batch 4-per-core, cast to
bf16, and pre-transpose context to [D, S] so the PE contraction dim (d)
lands on SBUF partitions without any on-chip transposes.

Per-core on-chip pipeline, per batch:
  - ctxT tiles [128d, 512s] arrive via DMA; W.T resident in SBUF.
  - pre[s,e] accumulated over 8 d-chunks into PSUM ([128s, 512e] x2).
  - ACT applies tanh PSUM->SBUF (bf16).
  - One DVE tensor_tensor_reduce: prod = h*v_bcast, accum_out = logits col.
  - softmax without max-subtraction (logits are tanh-bounded, |l| < ~6):
    ACT exp with fused accum -> per-partition sums; PE ones-matmul -> total;
    DVE reciprocal; PE broadcast matmul -> inv on all partitions.
  - weighted sum: PE accumulates exp(l) chunks against natural-layout ctx
    tiles, then DVE scales by inv(sum).
"""

import numpy as np
import ml_dtypes

import concourse.bass as bass
import concourse.mybir as mybir
import concourse.tile as tile
from concourse.bass_utils import run_bass_kernel_spmd


# ---------------------------------------------------------------------------
# This walrus build accepts at most ONE sync-wait command per instruction,
# while Tile emits several (e.g. the kernel-tail Drain waits on every sem
# lane). Hoist excess waits onto injected wait-only EventSemaphore
# instructions on the same engine, immediately before the instruction —
# strictly stronger ordering, so always safe.
import json as _json
import concourse.bass2jax as _b2j

_MAX_WAITS = 1
_orig_compile_bir_kernel = _b2j.compile_bir_kernel


def _split_excess_waits(bir, max_waits=_MAX_WAITS):
    bj = _json.loads(bir)
    changed = False
    for f in bj.get("functions", []):
        for bb in (f.get("basicblocks") or f.get("blocks") or []):
            out = []
            for ins in bb.get("instructions", []):
                si = ins.get("sync_info")
                if si:
                    w = si.get("on_wait") or []
                    if len(w) > max_waits:
                        for i, x in enumerate(w[:-max_waits]):
                            stub = {
                                "name": f"{ins['name']}-ws{i}",
                                "opcode": "EventSemaphore",
                                "engine": ins["engine"],
                                "ins": [],
                                "outs": [],
                                "sync_info": {"on_wait": [x], "on_update": []},
                            }
                            if "debug" in ins:
                                stub["debug"] = ins["debug"]
                            out.append(stub)
                        si["on_wait"] = w[-max_waits:]
                        changed = True
                out.append(ins)
            bb["instructions"] = out
    if not changed:
        return bir
    s = _json.dumps(bj)
    return s.encode() if isinstance(bir, bytes) else s


def _patched_compile_bir_kernel(bir_json, tmpdir, neff_name="file.neff"):
    return _orig_compile_bir_kernel(_split_excess_waits(bir_json), tmpdir, neff_name)


if _b2j.compile_bir_kernel is not _patched_compile_bir_kernel:
    _b2j.compile_bir_kernel = _patched_compile_bir_kernel
# ---------------------------------------------------------------------------

BF16 = mybir.dt.bfloat16
F32 = mybir.dt.float32
AF = mybir.ActivationFunctionType
ALU = mybir.AluOpType

B, S, D = 32, 2048, 1024
N_CORES = 8
BPC = B // N_CORES       # batches per core
P = 128
S_TILES = S // P         # 16
S_GROUPS = 4             # ctxT DMA groups per batch
SG = S // S_GROUPS       # 512
DC = D // P              # 8 contraction chunks
EH = 2                   # e halves of 512
NF = 512                 # matmul free dim / psum bank


def build_nc():
    nc = bass.Bass()
    ctxT = nc.declare_dram_parameter("ctxT", [BPC, D, S], BF16, isOutput=False)
    nat = nc.declare_dram_parameter("nat", [BPC, S, D], BF16, isOutput=False)
    wt = nc.declare_dram_parameter("wt", [D, D], BF16, isOutput=False)
    vb = nc.declare_dram_parameter("vb", [P, D], F32, isOutput=False)
    ones = nc.declare_dram_parameter("ones", [P, P], F32, isOutput=False)
    ident = nc.declare_dram_parameter("ident", [P, P], F32, isOutput=False)
    wc_out = nc.declare_dram_parameter("wc", [BPC, D], F32, isOutput=True)
    attn_out = nc.declare_dram_parameter("attn", [BPC, S], F32, isOutput=True)

    with tile.TileContext(nc) as tc:
        with (
            tc.tile_pool(name="const", bufs=1) as cpool,
            tc.tile_pool(name="ctxt", bufs=4) as ctxt_pool,
            tc.tile_pool(name="hbuf", bufs=3) as hpool,
            tc.tile_pool(name="prod", bufs=2) as prod_pool,
            tc.tile_pool(name="wsum", bufs=18) as wsum_pool,
            tc.tile_pool(name="small", bufs=2) as small_pool,
            tc.tile_pool(name="psum_pre", bufs=3, space="PSUM") as psum_pre,
            tc.tile_pool(name="psum_wc", bufs=2, space="PSUM") as psum_wc_pool,
            tc.tile_pool(name="psum_sm", bufs=1, space="PSUM") as psum_sm,
        ):
            wt_sb = cpool.tile([P, DC, D], BF16)
            wt_r = wt.rearrange("(dc p) e -> p dc e", p=P)

            # First ctxT group split per d-chunk and interleaved with the wt
            # chunks in consumption order, so MM(dc=0) can start after ~384KB
            # of DMA instead of 3MB.
            ctxT_first = ctxt_pool.tile([P, DC, SG], BF16, tag="ctxT_g")
            ctxT_first_r = ctxT[0, :, 0:SG].rearrange("(dc p) s -> p dc s", p=P)
            for dc in range(DC):
                nc.sync.dma_start(
                    ctxT_first[:, dc:dc + 1, :], ctxT_first_r[:, dc:dc + 1, :]
                )
                nc.scalar.dma_start(wt_sb[:, dc:dc + 1, :], wt_r[:, dc:dc + 1, :])
            vb_sb = cpool.tile([P, D], F32)
            nc.scalar.dma_start(vb_sb, vb[:])
            ones_sb = cpool.tile([P, P], F32)
            nc.scalar.dma_start(ones_sb, ones[:])
            ident_sb = cpool.tile([P, P], F32)
            nc.scalar.dma_start(ident_sb, ident[:])
            invpad = cpool.tile([P, BPC], F32)
            nc.gpsimd.memset(invpad, 0.0)

            # Small floor-cost PE warm-up that finishes before the first real
            # matmul's data lands: pushes HAM toward 8/8 without delaying the
            # real stream behind the FIFO queue.
            warm_src = cpool.tile([P, P], BF16)
            nc.vector.memset(warm_src, 0.0)
            warm_ps = psum_sm.tile([P, NF], F32, tag="sm")
            for _ in range(30):
                nc.tensor.matmul(warm_ps[:, 0:64], warm_src[:, 0:P], warm_src[:, 0:64])

            state = {}  # per-batch tiles needed by the deferred finalize

            def pre_phase(b):
                logits = small_pool.tile([P, S_TILES], F32, tag="logits")
                for g in range(S_GROUPS):
                    if b == 0 and g == 0:
                        ctxT_g = ctxT_first
                    else:
                        ctxT_g = ctxt_pool.tile([P, DC, SG], BF16, tag="ctxT_g")
                        nc.sync.dma_start(
                            ctxT_g,
                            ctxT[b, :, g * SG:(g + 1) * SG].rearrange(
                                "(dc p) s -> p dc s", p=P
                            ),
                        )
                    for t4 in range(SG // P):
                        t = g * (SG // P) + t4
                        ps = [
                            psum_pre.tile(
                                [P, NF], F32, tag="pre", name=f"pre_{b}_{t}_{eh}"
                            )
                            for eh in range(EH)
                        ]
                        for dc in range(DC):
                            lhsT = ctxT_g[:, dc, t4 * P:(t4 + 1) * P]
                            for eh in range(EH):
                                nc.tensor.matmul(
                                    ps[eh],
                                    lhsT,
                                    wt_sb[:, dc, eh * NF:(eh + 1) * NF],
                                    start=(dc == 0),
                                    stop=(dc == DC - 1),
                                )
                        h32 = hpool.tile([P, D], F32, tag="h32")
                        for eh in range(EH):
                            nc.scalar.activation(
                                h32[:, eh * NF:(eh + 1) * NF], ps[eh], AF.Tanh
                            )
                        prod = prod_pool.tile([P, D], F32, tag="prod")
                        # accum_out sums (h * v) in fp32 before `prod` rounding
                        nc.vector.scalar_tensor_tensor(
                            out=prod,
                            in0=h32,
                            scalar=1.0,
                            in1=vb_sb,
                            op0=ALU.mult,
                            op1=ALU.mult,
                            accum_out=logits[:, t:t + 1],
                        )
                # softmax pieces that don't touch PE: exp + bf16 cast
                E = small_pool.tile([P, S_TILES], F32, tag="E")
                sumexp = small_pool.tile([P, 1], F32, tag="sumexp")
                nc.scalar.activation(E, logits, AF.Exp, accum_out=sumexp)
                E16 = small_pool.tile([P, S_TILES], BF16, tag="E16")
                nc.vector.tensor_copy(E16, E)
                state[b] = (E, E16, sumexp)

            def finalize(b):
                E, E16, sumexp = state.pop(b)
                smps = psum_sm.tile([P, NF], F32, tag="sm")
                # total = sum over partitions of sumexp  -> smps[0,0]
                nc.tensor.matmul(smps[0:1, 0:1], sumexp, ones_sb[:, 0:1])
                nc.vector.reciprocal(invpad[0:1, b:b + 1], smps[0:1, 0:1])
                # broadcast inv to all 128 partitions -> smps[:,1]
                nc.tensor.matmul(smps[:, 1:2], ones_sb, invpad[:, b:b + 1])
                attn_f = small_pool.tile([P, S_TILES], F32, tag="attn_f")
                nc.vector.tensor_mul(
                    out=attn_f,
                    in0=E,
                    in1=smps[:, 1:2].to_broadcast([P, S_TILES]),
                )
                nc.tensor.transpose(smps[0:S_TILES, P:2 * P], attn_f, ident_sb)
                attnT = small_pool.tile([S_TILES, P], F32, tag="attnT")
                nc.vector.tensor_copy(attnT, smps[0:S_TILES, P:2 * P])
                nc.sync.dma_start(
                    attn_out[b].rearrange("(t p) -> t p", p=P), attnT
                )
                # weighted sum over s with unnormalized exp weights
                wcps = psum_wc_pool.tile([1, D], F32, tag="wc")
                for t in range(S_TILES):
                    nat_t = wsum_pool.tile([P, D], BF16, tag="nat")
                    nc.sync.dma_start(nat_t, nat[b, t * P:(t + 1) * P, :])
                    for eh in range(EH):
                        nc.tensor.matmul(
                            wcps[0:1, eh * NF:(eh + 1) * NF],
                            E16[:, t:t + 1],
                            nat_t[:, eh * NF:(eh + 1) * NF],
                            start=(t == 0),
                            stop=(t == S_TILES - 1),
                        )
                wc_sb = small_pool.tile([1, D], F32, tag="wc_sb")
                nc.vector.tensor_scalar_mul(wc_sb, wcps, invpad[0:1, b:b + 1])
                nc.sync.dma_start(wc_out[b][None, :], wc_sb)

            # Software-pipelined emission: batch b's softmax/weighted-sum work
            # (which stalls PE on an ACT->DVE chain) is emitted after batch
            # b+1's matmul stream, so the scheduler fills the stall with it.
            for b in range(BPC):
                pre_phase(b)
                if b > 0:
                    finalize(b - 1)
            finalize(BPC - 1)

    return nc


def _prep_in_maps(context, W, v):
    ctx16 = np.asarray(context, dtype=np.float32).astype(ml_dtypes.bfloat16)
    wt16 = np.ascontiguousarray(np.asarray(W, dtype=np.float32).T).astype(
        ml_dtypes.bfloat16
    )
    v32 = np.asarray(v, dtype=np.float32)
    vb32 = np.ascontiguousarray(np.broadcast_to(v32[None, :], (P, D)))
    ones = np.ones((P, P), dtype=np.float32)
    ident = np.eye(P, dtype=np.float32)

    in_maps = []
    for c in range(N_CORES):
        shard = ctx16[c * BPC:(c + 1) * BPC]            # [BPC, S, D] bf16
        ctxT = np.ascontiguousarray(shard.transpose(0, 2, 1))  # [BPC, D, S]
        in_maps.append(
            {
                "ctxT": ctxT,
                "nat": np.ascontiguousarray(shard),
                "wt": wt16,
                "vb": vb32,
                "ones": ones,
                "ident": ident,
            }
        )
    return in_maps


def run(context, W, v, b=None, trace=False, **trace_kwargs):
    nc = build_nc()
    in_maps = _prep_in_maps(context, W, v)
    res = run_bass_kernel_spmd(
        nc, in_maps, list(range(N_CORES)), trace=trace, **trace_kwargs
    )
    wc = np.concatenate([r["wc"] for r in res.results], axis=0)
    attn = np.concatenate([r["attn"] for r in res.results], axis=0)
    return (wc, attn), res


def kernel(context, W, v, b=None, **_ignored):
    out, _ = run(context, W, v, b)
    return out


# revision 15
# speedup vs baseline: 1.1173x; 1.1005x over previous
"""Attention-simple kernel for TRN2, 8 NeuronCores, data-parallel over batch.

Computation (per example b):
    h = tanh(context[b] @ W.T)          # [S, D]
    logits = h @ v (+ bias, dropped: softmax-invariant)
    attn = softmax(logits)              # [S]
    wc = attn @ context[b]              # [D]
returns (wc [B, D], attn [B, S])

Host-side prep (not counted in HW time): shard batch 4-per-core, cast to
bf16, and pre-transpose context to [D, S] so the PE contraction dim (d)
lands on SBUF partitions without any on-chip transposes.

Per-core on-chip pipeline, per batch:
  - ctxT tiles [128d, 512s] arrive via DMA; W.T resident in SBUF.
  - pre[s,e] accumulated over 8 d-chunks into PSUM ([128s, 512e] x2).
  - ACT applies tanh PSUM->SBUF (bf16).
  - One DVE tensor_tensor_reduce: prod = h*v_bcast, accum_out = logits col.
  - softmax without max-subtraction (logits are tanh-bounded, |l| < ~6):
    ACT exp with fused accum -> per-partition sums; PE ones-matmul -> total;
    DVE reciprocal; PE broadcast matmul -> inv on all partitions.
  - weighted sum: PE accumulates exp(l) chunks against natural-layout ctx
    tiles, then DVE scales by inv(sum).
"""

import numpy as np
import ml_dtypes

import concourse.bass as bass
import concourse.mybir as mybir
import concourse.tile as tile
from concourse.bass_utils import run_bass_kernel_spmd


# ---------------------------------------------------------------------------
# This walrus build accepts at most ONE sync-wait command per instruction,
# while Tile emits several (e.g. the kernel-tail Drain waits on every sem
# lane). Hoist excess waits onto injected wait-only EventSemaphore
# instructions on the same engine, immediately before the instruction —
# strictly stronger ordering, so always safe.
import json as _json
import concourse.bass2jax as _b2j

_MAX_WAITS = 1
_orig_compile_bir_kernel = _b2j.compile_bir_kernel


def _split_excess_waits(bir, max_waits=_MAX_WAITS):
    bj = _json.loads(bir)
    changed = False
    for f in bj.get("functions", []):
        for bb in (f.get("basicblocks") or f.get("blocks") or []):
            out = []
            for ins in bb.get("instructions", []):
                si = ins.get("sync_info")
                if si:
                    w = si.get("on_wait") or []
                    if len(w) > max_waits:
                        for i, x in enumerate(w[:-max_waits]):
                            stub = {
                                "name": f"{ins['name']}-ws{i}",
                                "opcode": "EventSemaphore",
                                "engine": ins["engine"],
                                "ins": [],
                                "outs": [],
                                "sync_info": {"on_wait": [x], "on_update": []},
                            }
                            if "debug" in ins:
                                stub["debug"] = ins["debug"]
                            out.append(stub)
                        si["on_wait"] = w[-max_waits:]
                        changed = True
                out.append(ins)
            bb["instructions"] = out
    if not changed:
        return bir
    s = _json.dumps(bj)
    return s.encode() if isinstance(bir, bytes) else s


def _patched_compile_bir_kernel(bir_json, tmpdir, neff_name="file.neff"):
    return _orig_compile_bir_kernel(_split_excess_waits(bir_json), tmpdir, neff_name)


if _b2j.compile_bir_kernel is not _patched_compile_bir_kernel:
    _b2j.compile_bir_kernel = _patched_compile_bir_kernel
# ---------------------------------------------------------------------------

BF16 = mybir.dt.bfloat16
F32 = mybir.dt.float32
AF = mybir.ActivationFunctionType
ALU = mybir.AluOpType

B, S, D = 32, 2048, 1024
N_CORES = 8
BPC = B // N_CORES       # batches per core
P = 128
S_TILES = S // P         # 16
S_GROUPS = 4             # ctxT DMA groups per batch
SG = S // S_GROUPS       # 512
DC = D // P              # 8 contraction chunks
EH = 2                   # e halves of 512
NF = 512                 # matmul free dim / psum bank


def build_nc():
    nc = bass.Bass()
    ctxT = nc.declare_dram_parameter("ctxT", [BPC, D, S], BF16, isOutput=False)
    nat = nc.declare_dram_parameter("nat", [BPC, S, D], BF16, isOutput=False)
    wt = nc.declare_dram_parameter("wt", [D, D], BF16, isOutput=False)
    vb = nc.declare_dram_parameter("vb", [P, D], F32, isOutput=False)
    ones = nc.declare_dram_parameter("ones", [P, P], F32, isOutput=False)
    ident = nc.declare_dram_parameter("ident", [P, P], F32, isOutput=False)
    wc_out = nc.declare_dram_parameter("wc", [BPC, D], F32, isOutput=True)
    attn_out = nc.declare_dram_parameter("attn", [BPC, S], F32, isOutput=True)

    with tile.TileContext(nc) as tc:
        with (
            tc.tile_pool(name="const", bufs=1) as cpool,
            tc.tile_pool(name="ctxt", bufs=4) as ctxt_pool,
            tc.tile_pool(name="hbuf", bufs=3) as hpool,
            tc.tile_pool(name="prod", bufs=2) as prod_pool,
            tc.tile_pool(name="wsum", bufs=6) as wsum_pool,
            tc.tile_pool(name="acc", bufs=2) as acc_pool,
            tc.tile_pool(name="small", bufs=2) as small_pool,
            tc.tile_pool(name="psum_pre", bufs=3, space="PSUM") as psum_pre,
            tc.tile_pool(name="psum_wc", bufs=2, space="PSUM") as psum_wc_pool,
            tc.tile_pool(name="psum_sm", bufs=1, space="PSUM") as psum_sm,
        ):
            wt_sb = cpool.tile([P, DC, D], BF16)
            wt_r = wt.rearrange("(dc p) e -> p dc e", p=P)

            # First ctxT group split per d-chunk and interleaved with the wt
            # chunks in consumption order, so MM(dc=0) can start after ~384KB
            # of DMA instead of 3MB.
            ctxT_first = ctxt_pool.tile([P, DC, SG], BF16, tag="ctxT_g")
            ctxT_first_r = ctxT[0, :, 0:SG].rearrange("(dc p) s -> p dc s", p=P)
            for dc in range(DC):
                nc.sync.dma_start(
                    ctxT_first[:, dc:dc + 1, :], ctxT_first_r[:, dc:dc + 1, :]
                )
                nc.scalar.dma_start(wt_sb[:, dc:dc + 1, :], wt_r[:, dc:dc + 1, :])
            vb_sb = cpool.tile([P, D], F32)
            nc.scalar.dma_start(vb_sb, vb[:])
            ones_sb = cpool.tile([P, P], F32)
            nc.scalar.dma_start(ones_sb, ones[:])
            ident_sb = cpool.tile([P, P], F32)
            nc.scalar.dma_start(ident_sb, ident[:])
            invpad = cpool.tile([P, BPC], F32)
            nc.gpsimd.memset(invpad, 0.0)

            # Small floor-cost PE warm-up that finishes before the first real
            # matmul's data lands: pushes HAM toward 8/8 without delaying the
            # real stream behind the FIFO queue.
            warm_src = cpool.tile([P, P], BF16)
            nc.vector.memset(warm_src, 0.0)
            warm_ps = psum_sm.tile([P, NF], F32, tag="sm")
            for _ in range(30):
                nc.tensor.matmul(warm_ps[:, 0:64], warm_src[:, 0:P], warm_src[:, 0:64])

            # Weighted sum runs on DVE: per s-tile, acc += nat_t * exp(logit)
            # with the exp column as a per-partition fp32 scalar. PE only does
            # the final cross-partition ones-matmul per batch. The exp/acc ops
            # for tile t are emitted after tile t+1's dot-product so ACT and
            # DVE pipeline across tiles instead of serializing.
            state = {}      # b -> (logits, E, sumexp, acc)
            pending = []    # [(b, t)] exp+acc work not yet emitted

            def emit_acc(b, t):
                logits, E, _, acc = state[b]
                nc.scalar.activation(
                    E[:, t:t + 1], logits[:, t:t + 1], AF.Exp
                )
                nat_t = wsum_pool.tile([P, D], BF16, tag="nat")
                nc.sync.dma_start(nat_t, nat[b, t * P:(t + 1) * P, :])
                if t == 0:
                    nc.vector.tensor_scalar_mul(acc, nat_t, E[:, 0:1])
                else:
                    nc.vector.scalar_tensor_tensor(
                        out=acc,
                        in0=nat_t,
                        scalar=E[:, t:t + 1],
                        in1=acc,
                        op0=ALU.mult,
                        op1=ALU.add,
                    )

            def drain_pending(keep):
                while len(pending) > keep:
                    emit_acc(*pending.pop(0))

            def pre_phase(b):
                state[b] = (
                    small_pool.tile([P, S_TILES], F32, tag="logits", name=f"lg{b}"),
                    small_pool.tile([P, S_TILES], F32, tag="E", name=f"E{b}"),
                    small_pool.tile([P, 1], F32, tag="sumexp", name=f"se{b}"),
                    acc_pool.tile([P, D], F32, tag="acc", name=f"acc{b}"),
                )
                logits = state[b][0]
                for g in range(S_GROUPS):
                    if b == 0 and g == 0:
                        ctxT_g = ctxT_first
                    else:
                        ctxT_g = ctxt_pool.tile([P, DC, SG], BF16, tag="ctxT_g")
                        nc.sync.dma_start(
                            ctxT_g,
                            ctxT[b, :, g * SG:(g + 1) * SG].rearrange(
                                "(dc p) s -> p dc s", p=P
                            ),
                        )
                    for t4 in range(SG // P):
                        t = g * (SG // P) + t4
                        ps = [
                            psum_pre.tile(
                                [P, NF], F32, tag="pre", name=f"pre_{b}_{t}_{eh}"
                            )
                            for eh in range(EH)
                        ]
                        for dc in range(DC):
                            lhsT = ctxT_g[:, dc, t4 * P:(t4 + 1) * P]
                            for eh in range(EH):
                                nc.tensor.matmul(
                                    ps[eh],
                                    lhsT,
                                    wt_sb[:, dc, eh * NF:(eh + 1) * NF],
                                    start=(dc == 0),
                                    stop=(dc == DC - 1),
                                )
                        h32 = hpool.tile([P, D], F32, tag="h32")
                        for eh in range(EH):
                            nc.scalar.activation(
                                h32[:, eh * NF:(eh + 1) * NF], ps[eh], AF.Tanh
                            )
                        prod = prod_pool.tile([P, D], F32, tag="prod")
                        # accum_out sums (h * v) in fp32 before `prod` rounding
                        nc.vector.scalar_tensor_tensor(
                            out=prod,
                            in0=h32,
                            scalar=1.0,
                            in1=vb_sb,
                            op0=ALU.mult,
                            op1=ALU.mult,
                            accum_out=logits[:, t:t + 1],
                        )
                        pending.append((b, t))
                        drain_pending(1)

            def finalize(b):
                logits, E, sumexp, acc = state.pop(b)
                nc.vector.reduce_sum(sumexp, E, axis=mybir.AxisListType.X)
                smps = psum_sm.tile([P, NF], F32, tag="sm")
                # total = sum over partitions of sumexp  -> smps[0,0]
                nc.tensor.matmul(smps[0:1, 0:1], sumexp, ones_sb[:, 0:1])
                nc.vector.reciprocal(invpad[0:1, b:b + 1], smps[0:1, 0:1])
                # broadcast inv to all 128 partitions -> smps[:,1]
                nc.tensor.matmul(smps[:, 1:2], ones_sb, invpad[:, b:b + 1])
                attn_f = small_pool.tile([P, S_TILES], F32, tag="attn_f")
                nc.vector.tensor_mul(
                    out=attn_f,
                    in0=E,
                    in1=smps[:, 1:2].to_broadcast([P, S_TILES]),
                )
                nc.tensor.transpose(smps[0:S_TILES, P:2 * P], attn_f, ident_sb)
                attnT = small_pool.tile([S_TILES, P], F32, tag="attnT")
                nc.vector.tensor_copy(attnT, smps[0:S_TILES, P:2 * P])
                nc.sync.dma_start(
                    attn_out[b].rearrange("(t p) -> t p", p=P), attnT
                )
                # cross-partition finish of the weighted sum: fp32 ones-matmul
                wcps = psum_wc_pool.tile([1, D], F32, tag="wc")
                for eh in range(EH):
                    nc.tensor.matmul(
                        wcps[0:1, eh * NF:(eh + 1) * NF],
                        ones_sb[:, 0:1],
                        acc[:, eh * NF:(eh + 1) * NF],
                    )
                wc_sb = small_pool.tile([1, D], F32, tag="wc_sb")
                nc.vector.tensor_scalar_mul(wc_sb, wcps, invpad[0:1, b:b + 1])
                nc.sync.dma_start(wc_out[b][None, :], wc_sb)

            for b in range(BPC):
                pre_phase(b)
                drain_pending(1 if b < BPC - 1 else 0)
                if b > 0:
                    finalize(b - 1)
            finalize(BPC - 1)

    return nc


def _prep_in_maps(context, W, v):
    ctx16 = np.asarray(context, dtype=np.float32).astype(ml_dtypes.bfloat16)
    wt16 = np.ascontiguousarray(np.asarray(W, dtype=np.float32).T).astype(
        ml_dtypes.bfloat16
    )
    v32 = np.asarray(v, dtype=np.float32)
    vb32 = np.ascontiguousarray(np.broadcast_to(v32[None, :], (P, D)))
    ones = np.ones((P, P), dtype=np.float32)
    ident = np.eye(P, dtype=np.float32)

    in_maps = []
    for c in range(N_CORES):
        shard = ctx16[c * BPC:(c + 1) * BPC]            # [BPC, S, D] bf16
        ctxT = np.ascontiguousarray(shard.transpose(0, 2, 1))  # [BPC, D, S]
        in_maps.append(
            {
                "ctxT": ctxT,
                "nat": np.ascontiguousarray(shard),
                "wt": wt16,
                "vb": vb32,
                "ones": ones,
                "ident": ident,
            }
        )
    return in_maps


def run(context, W, v, b=None, trace=False, **trace_kwargs):
    nc = build_nc()
    in_maps = _prep_in_maps(context, W, v)
    res = run_bass_kernel_spmd(
        nc, in_maps, list(range(N_CORES)), trace=trace, **trace_kwargs
    )
    wc = np.concatenate([r["wc"] for r in res.results], axis=0)
    attn = np.concatenate([r["attn"] for r in res.results], axis=0)
    return (wc, attn), res


def kernel(context, W, v, b=None, **_ignored):
    out, _ = run(context, W, v, b)
    return out
